# revision 1
# baseline (speedup 1.0000x reference)
"""Trainium2 Bass kernel for nn_AtomicLinear: out = x @ W.T + bias.

Shapes (hardcoded): x (4096, 2048) f32, weight (2048, 2048) f32 [out, in],
bias (2048,) f32 -> out (4096, 2048) f32.

Sharding across 8 NeuronCores: 2D grid of 4 batch-groups x 2 out-feature
groups. Core c handles batch rows [bg*1024, (bg+1)*1024) and out features
[og*1024, (og+1)*1024) with bg = c // 2, og = c % 2. Per-core HBM traffic is
8 MB (x^T shard) + 8 MB (W^T shard) + 0.5 MB (bias bcast) + 4 MB (out) ~=
20.5 MB -- less than pure data-parallel (24 MB), and balanced against the
~55 us fp32r TensorE time (true ridge).

The TensorE contracts over the partition dim, so both operands need
in_features on partitions; fp32 has no DMA-transpose path, so the wrapper
marshals x^T / W^T (host-side layout choice during sharding) and the device
kernel is a pure fp32r matmul + bias add:

  psum[b(128), o(512)] += sum_k xT[k, b-slice] (lhsT) @ wT[k, o-slice] (rhs)

fp32r streams 1 row/cycle at free-dim >= 256 (vs 4 cycles/row for exact
fp32), accumulating in fp32 PSUM.
"""

import numpy as np

BATCH = 4096
IN_F = 2048
OUT_F = 2048
N_CORES = 8
BG = 4  # batch groups
OG = 2  # out-feature groups
B_SH = BATCH // BG  # 1024 batch rows per core
O_SH = OUT_F // OG  # 1024 out features per core
P = 128
N_TILE = 512
K_TILES = IN_F // P  # 16
M_TILES = B_SH // P  # 8
N_TILES = O_SH // N_TILE  # 2

_BUILT = None  # cached (nc, names) -- neuronx compile is expensive


def _build():
    import concourse.mybir as mybir
    import concourse.tile as tile
    from concourse import bacc

    nc = bacc.Bacc(None, target_bir_lowering=False, debug=False)

    xT = nc.declare_dram_parameter("xT", [IN_F, B_SH], mybir.dt.float32r, isOutput=False)
    wT = nc.declare_dram_parameter("wT", [IN_F, O_SH], mybir.dt.float32r, isOutput=False)
    bias_b = nc.declare_dram_parameter("bias_b", [P, O_SH], mybir.dt.float32, isOutput=False)
    out = nc.declare_dram_parameter("out", [B_SH, O_SH], mybir.dt.float32, isOutput=True)

    with tile.TileContext(nc) as tc:
        with (
            tc.tile_pool(name="persist", bufs=1) as persist,
            tc.tile_pool(name="ot_pool", bufs=4) as ot_pool,
            tc.tile_pool(name="ps_pool", bufs=8, space="PSUM") as ps_pool,
        ):
            bias_sb = persist.tile([P, O_SH], mybir.dt.float32, name="bias_sb", tag="bias_sb")
            nc.sync.dma_start(out=bias_sb[:], in_=bias_b[:])

            # One SBUF tile per 128-row k-chunk of each operand: fine-grained
            # DMA->matmul dependencies so compute starts after chunk 0 lands.
            # Everything stays resident (8 MB + 8 MB < 24 MB SBUF).
            wk = []
            xk = []
            for k in range(K_TILES):
                wt = persist.tile([P, O_SH], mybir.dt.float32r, name=f"wk{k}", tag=f"wk{k}")
                nc.sync.dma_start(out=wt[:], in_=wT[k * P : (k + 1) * P, :])
                wk.append(wt)
                xt = persist.tile([P, B_SH], mybir.dt.float32r, name=f"xk{k}", tag=f"xk{k}")
                nc.sync.dma_start(out=xt[:], in_=xT[k * P : (k + 1) * P, :])
                xk.append(xt)

            for m in range(M_TILES):
                psums = []
                for n in range(N_TILES):
                    pt = ps_pool.tile([P, N_TILE], mybir.dt.float32, name=f"ps_{m}_{n}", tag="ps")
                    psums.append(pt)
                for k in range(K_TILES):
                    lhsT = xk[k][:, m * P : (m + 1) * P]
                    for n in range(N_TILES):
                        nc.tensor.matmul(
                            psums[n][:],
                            lhsT,
                            wk[k][:, n * N_TILE : (n + 1) * N_TILE],
                            start=(k == 0),
                            stop=(k == K_TILES - 1),
                        )
                for n in range(N_TILES):
                    ot = ot_pool.tile([P, N_TILE], mybir.dt.float32, name=f"ot_{m}_{n}", tag="ot")
                    nc.vector.tensor_add(
                        out=ot[:],
                        in0=psums[n][:],
                        in1=bias_sb[:, n * N_TILE : (n + 1) * N_TILE],
                    )
                    # ACT-ring DMA so output writes don't queue behind the
                    # input reads on the SP HWDGE ring.
                    nc.scalar.dma_start(
                        out=out[m * P : (m + 1) * P, n * N_TILE : (n + 1) * N_TILE],
                        in_=ot[:],
                    )

    nc.compile()
    return nc


def _get_built():
    global _BUILT
    if _BUILT is None:
        _BUILT = _build()
    return _BUILT


def _make_in_maps(x, weight, bias):
    x = np.ascontiguousarray(x, dtype=np.float32)
    weight = np.ascontiguousarray(weight, dtype=np.float32)
    bias = np.ascontiguousarray(bias, dtype=np.float32)

    xT_q = [np.ascontiguousarray(x[bg * B_SH : (bg + 1) * B_SH, :].T) for bg in range(BG)]
    wT_h = [np.ascontiguousarray(weight[og * O_SH : (og + 1) * O_SH, :].T) for og in range(OG)]
    bias_bc = [
        np.ascontiguousarray(np.broadcast_to(bias[og * O_SH : (og + 1) * O_SH], (P, O_SH)))
        for og in range(OG)
    ]

    in_maps = []
    for c in range(N_CORES):
        bg, og = c // OG, c % OG
        in_maps.append({"xT": xT_q[bg], "wT": wT_h[og], "bias_b": bias_bc[og]})
    return in_maps


def _assemble(results):
    full = np.empty((BATCH, OUT_F), dtype=np.float32)
    for c in range(N_CORES):
        bg, og = c // OG, c % OG
        full[bg * B_SH : (bg + 1) * B_SH, og * O_SH : (og + 1) * O_SH] = results[c]["out"]
    return full


def _run(inputs, trace=False, **spmd_kwargs):
    """Run the SPMD kernel; returns (full_output, BassKernelResults)."""
    from concourse.bass_utils import run_bass_kernel_spmd

    nc = _get_built()
    in_maps = _make_in_maps(inputs["x"], inputs["weight"], inputs["bias"])
    res = run_bass_kernel_spmd(nc, in_maps, list(range(N_CORES)), trace=trace, **spmd_kwargs)
    return _assemble(res.results), res


def kernel(x, weight, bias):
    out, _ = _run({"x": x, "weight": weight, "bias": bias})
    return out
